# revision 8
# baseline (speedup 1.0000x reference)
"""Trainium2 Bass kernel for AtomMPNN (gnn_message_passing).

Full inputs in, full outputs out. Internally: 8 NeuronCores, one (sample,
node-half) shard per core. See build_nc() for the device-side pipeline.

Self-contained: hardcodes all shapes; only imports the concourse runtime.
"""

import sys

sys.path.insert(0, "/opt/trn_rl_repo")

import numpy as np
import ml_dtypes

import concourse.bass as bass
import concourse.tile as tile
from concourse import bacc, mybir
from concourse import library_config
from concourse.bass_utils import run_bass_kernel_spmd

B, N, K, H = 4, 4096, 32, 128
NODES = N // 2  # own nodes per core
NCORES = 8
EPS = 1e-5
NEG_BIG = -30000.0  # forces gelu(x) == 0 for padded edges

F32 = mybir.dt.float32
BF16 = mybir.dt.bfloat16
I16 = mybir.dt.int16

BF = ml_dtypes.bfloat16


def build_nc(num_cores=NCORES, n_all=N, nodes=NODES, k=K, h=H):
    """Build the SPMD Bass graph (same program on every core)."""
    nc = bacc.Bacc(
        "TRN2", target_bir_lowering=False, debug=False, num_devices=num_cores
    )

    # ---- DRAM parameters (per-core shards; host-prepped layouts) ----
    encT = nc.declare_dram_parameter("encT", [h, n_all], F32, isOutput=False)
    maskr = nc.declare_dram_parameter("maskr", [1, n_all], F32, isOutput=False)
    dist32 = nc.declare_dram_parameter("dist32", [k, nodes], BF16, isOutput=False)
    inv32 = nc.declare_dram_parameter("inv32", [k, nodes], BF16, isOutput=False)
    idx16 = nc.declare_dram_parameter("idx16", [128, k * (nodes // 16)], I16, isOutput=False)
    w0a = nc.declare_dram_parameter("w0a", [h, h], BF16, isOutput=False)
    w0b = nc.declare_dram_parameter("w0b", [h, h], BF16, isOutput=False)
    w1 = nc.declare_dram_parameter("w1", [h, h], BF16, isOutput=False)
    w2 = nc.declare_dram_parameter("w2", [h, h], BF16, isOutput=False)
    seld = nc.declare_dram_parameter("seld", [k, k * h], BF16, isOutput=False)
    seli = nc.declare_dram_parameter("seli", [k, k * h], BF16, isOutput=False)
    ident = nc.declare_dram_parameter("ident", [h, h], BF16, isOutput=False)
    onesk = nc.declare_dram_parameter("onesk", [k, h], BF16, isOutput=False)
    b0 = nc.declare_dram_parameter("b0", [h, 1], F32, isOutput=False)
    b1 = nc.declare_dram_parameter("b1", [h, 1], F32, isOutput=False)
    b2 = nc.declare_dram_parameter("b2", [h, 1], F32, isOutput=False)
    gscale = nc.declare_dram_parameter("gscale", [h, 1], F32, isOutput=False)
    gshift = nc.declare_dram_parameter("gshift", [h, 1], F32, isOutput=False)
    out_ext = nc.declare_dram_parameter("out", [h, nodes], F32, isOutput=True)

    ew = min(512, nodes)  # matmul free-dim slice width
    nsl = nodes // ew  # slices per block
    nchunk = n_all // 128  # 128-col chunks for the gather table
    pair_groups = [[2 * i, 2 * i + 1] for i in range(num_cores // 2)]
    gelu = mybir.ActivationFunctionType.Gelu
    identity_f = mybir.ActivationFunctionType.Identity
    square_f = mybir.ActivationFunctionType.Square
    sqrt_f = mybir.ActivationFunctionType.Sqrt
    add_op = mybir.AluOpType.add
    mult_op = mybir.AluOpType.mult

    with tile.TileContext(nc) as tc:
        with (
            tc.tile_pool(name="const", bufs=1) as cpool,
            tc.tile_pool(name="stat", bufs=1) as spool,
            tc.tile_pool(name="work", bufs=3) as wpool,
            tc.tile_pool(name="psum", bufs=2, space="PSUM") as ppool,
            tc.tile_pool(name="dram", bufs=1, space="DRAM") as dpool,
        ):
            # dma_gather lives in the 'mlp' GPSIMD ext-isa library
            nc.gpsimd.load_library(library_config.mlp)

            # ---- load constants / weights ----
            w0a_sb = cpool.tile([h, h], BF16)
            w0b_sb = cpool.tile([h, h], BF16)
            w1_sb = cpool.tile([h, h], BF16)
            w2_sb = cpool.tile([h, h], BF16)
            id_sb = cpool.tile([h, h], BF16)
            seld_sb = cpool.tile([k, k * h], BF16)
            seli_sb = cpool.tile([k, k * h], BF16)
            onesk_sb = cpool.tile([k, h], BF16)
            b0_sb = cpool.tile([h, 1], F32)
            b1_sb = cpool.tile([h, 1], F32)
            b2_sb = cpool.tile([h, 1], F32)
            gsc_sb = cpool.tile([h, 1], F32)
            gsh_sb = cpool.tile([h, 1], F32)
            for dst, src in [
                (w0a_sb, w0a), (w0b_sb, w0b), (w1_sb, w1), (w2_sb, w2),
                (id_sb, ident), (seld_sb, seld), (seli_sb, seli),
                (onesk_sb, onesk), (b0_sb, b0), (b1_sb, b1), (b2_sb, b2),
                (gsc_sb, gscale), (gsh_sb, gshift),
            ]:
                nc.sync.dma_start(out=dst[:], in_=src[:])

            # ---- load per-core data ----
            encT_sb = spool.tile([h, n_all], F32)
            nc.sync.dma_start(out=encT_sb[:], in_=encT[:])
            dist_sb = spool.tile([k, nodes], BF16)
            nc.sync.dma_start(out=dist_sb[:], in_=dist32[:])
            inv_sb = spool.tile([k, nodes], BF16)
            nc.sync.dma_start(out=inv_sb[:], in_=inv32[:])
            idx_sb = spool.tile([128, k * (nodes // 16)], I16)
            nc.sync.dma_start(out=idx_sb[:], in_=idx16[:])
            # mask broadcast to all 128 partitions (stride-0 partition read)
            m_rep = spool.tile([128, n_all], F32)
            mask_bcast = bass.AP(
                tensor=maskr.tensor if hasattr(maskr, "tensor") else maskr[:].tensor,
                offset=maskr[:].offset,
                ap=[[0, 128]] + list(maskr[:].ap[1:]),
            )
            nc.gpsimd.dma_start(out=m_rep[:], in_=mask_bcast)

            # ---- masked features (bf16): enc_m = encT * mask ----
            enc_m = spool.tile([h, n_all], BF16)
            nc.vector.tensor_mul(enc_m[:], encT_sb[:], m_rep[:])

            # ---- gather table P[i,:] = enc_m[:,i] @ W0a  (node-major rows) ----
            # node i -> partition i%128, stripe i//128 (256B bf16 stripes)
            ptab = spool.tile([128, nchunk * h], BF16)
            for q in range(nchunk):
                ps = ppool.tile([128, 2048], F32, tag="ps")
                nc.tensor.matmul(
                    ps[:, 0:h],
                    lhsT=enc_m[:, q * 128 : (q + 1) * 128],
                    rhs=w0a_sb[:],
                    start=True,
                    stop=True,
                )
                nc.vector.tensor_copy(
                    out=ptab[:, q * h : (q + 1) * h], in_=ps[:, 0:h]
                )

            # ---- self term S'' = W0b^T @ enc_m[:, :nodes] + b0 (bf16) ----
            s2_sb = spool.tile([h, nodes], BF16)
            ps_s = ppool.tile([128, 2048], F32, tag="ps")
            for q in range(nsl):
                nc.tensor.matmul(
                    ps_s[:, q * ew : (q + 1) * ew],
                    lhsT=w0b_sb[:],
                    rhs=enc_m[:, q * ew : (q + 1) * ew],
                    start=True,
                    stop=True,
                )
            nc.scalar.activation(
                out=s2_sb[:], in_=ps_s[:, 0:nodes], func=identity_f, bias=b0_sb[:]
            )

            # ---- valid-neighbor count, replicated: vcnt_inv[c,n] = 1/max(k - sum_j inv, 1)
            ps_v = ppool.tile([128, 2048], F32, tag="ps")
            for q in range(nsl):
                nc.tensor.matmul(
                    ps_v[:, q * ew : (q + 1) * ew],
                    lhsT=onesk_sb[:],
                    rhs=inv_sb[:, q * ew : (q + 1) * ew],
                    start=True,
                    stop=True,
                )
            vcnt = spool.tile([h, nodes], F32)
            nc.vector.tensor_scalar(
                out=vcnt[:], in0=ps_v[:, 0:nodes], scalar1=-1.0, scalar2=float(k),
                op0=mult_op, op1=add_op,
            )
            nc.vector.tensor_scalar_max(vcnt[:], vcnt[:], 1.0)
            vinv = spool.tile([h, nodes], F32)
            nc.vector.reciprocal(vinv[:], vcnt[:])

            # ---- edge blocks: block k' = neighbor slot k' for all own nodes ----
            msum = spool.tile([h, nodes], BF16)
            for kb in range(k):
                g_t = wpool.tile([128, 1, nodes], BF16, tag="g")
                nc.gpsimd.dma_gather(
                    out_ap=g_t[:],
                    in_ap=ptab[:],
                    idxs_ap=idx_sb[:, kb * (nodes // 16) : (kb + 1) * (nodes // 16)],
                    num_idxs=nodes,
                    num_idxs_reg=nodes,
                    elem_size=h,
                    transpose=True,
                    sbuf_tokens_per_rank=128,
                    sbuf_free_dim_per_rank=2 * h,
                    single_packet=False,
                )
                # t1 = gathered + self
                t1 = wpool.tile([h, nodes], BF16, tag="t1")
                nc.vector.tensor_add(t1[:], g_t[:, 0, :], s2_sb[:])
                # a0 = t1 + dist*w0d  (+b0 already inside s2)
                ps_a = ppool.tile([128, 2048], F32, tag="ps")
                for q in range(nsl):
                    sl = slice(q * ew, (q + 1) * ew)
                    nc.tensor.matmul(
                        ps_a[:, sl],
                        lhsT=seld_sb[:, kb * h : (kb + 1) * h],
                        rhs=dist_sb[:, sl],
                        start=True,
                        stop=False,
                    )
                    nc.tensor.matmul(
                        ps_a[:, sl], lhsT=id_sb[:], rhs=t1[:, sl],
                        start=False, stop=True,
                    )
                h0 = wpool.tile([h, nodes], BF16, tag="h0")
                nc.scalar.activation(out=h0[:], in_=ps_a[:, 0:nodes], func=gelu)
                # layer 1
                ps_b = ppool.tile([128, 2048], F32, tag="ps")
                for q in range(nsl):
                    sl = slice(q * ew, (q + 1) * ew)
                    nc.tensor.matmul(
                        ps_b[:, sl], lhsT=w1_sb[:], rhs=h0[:, sl],
                        start=True, stop=True,
                    )
                h1 = wpool.tile([h, nodes], BF16, tag="h1")
                nc.scalar.activation(
                    out=h1[:], in_=ps_b[:, 0:nodes], func=gelu, bias=b1_sb[:]
                )
                # layer 2 (+ -30000 * invalid so gelu masks padded edges)
                ps_c = ppool.tile([128, 2048], F32, tag="ps")
                for q in range(nsl):
                    sl = slice(q * ew, (q + 1) * ew)
                    nc.tensor.matmul(
                        ps_c[:, sl], lhsT=w2_sb[:], rhs=h1[:, sl],
                        start=True, stop=False,
                    )
                    nc.tensor.matmul(
                        ps_c[:, sl],
                        lhsT=seli_sb[:, kb * h : (kb + 1) * h],
                        rhs=inv_sb[:, sl],
                        start=False,
                        stop=True,
                    )
                mk = wpool.tile([h, nodes], BF16, tag="mk")
                nc.scalar.activation(
                    out=mk[:], in_=ps_c[:, 0:nodes], func=gelu, bias=b2_sb[:]
                )
                if kb == 0:
                    nc.vector.tensor_copy(out=msum[:], in_=mk[:])
                else:
                    nc.vector.tensor_add(msum[:], msum[:], mk[:])

            # ---- node phase: upd = (enc + msum/vcnt) * mask ----
            msum_f = spool.tile([h, nodes], F32)
            nc.vector.tensor_copy(out=msum_f[:], in_=msum[:])
            upd = spool.tile([h, nodes], F32)
            nc.vector.tensor_mul(upd[:], msum_f[:], vinv[:])
            nc.vector.tensor_add(upd[:], upd[:], encT_sb[:, 0:nodes])
            nc.vector.tensor_mul(upd[:], upd[:], m_rep[:, 0:nodes])

            # ---- partial stats: S1 = sum(upd), S2 = sum(upd^2), C = sum(mask)
            stats = spool.tile([h, 4], F32)
            nc.vector.tensor_reduce(
                out=stats[:, 0:1], in_=upd[:], axis=mybir.AxisListType.X, op=add_op
            )
            sq_trash = spool.tile([h, nodes], BF16)
            nc.scalar.activation(
                out=sq_trash[:], in_=upd[:], func=square_f,
                accum_out=stats[:, 1:2],
            )
            nc.vector.tensor_reduce(
                out=stats[:, 2:3], in_=m_rep[:, 0:nodes],
                axis=mybir.AxisListType.X, op=add_op,
            )
            nc.vector.memset(stats[:, 3:4], 0.0)

            # ---- pair all-reduce of stats (both halves of the sample) ----
            cc_in = dpool.tile([h, 4], F32)
            cc_out = dpool.tile([h, 4], F32)
            nc.gpsimd.dma_start(out=cc_in[:], in_=stats[:])
            nc.gpsimd.collective_compute(
                "AllReduce",
                mybir.AluOpType.add,
                replica_groups=pair_groups,
                ins=[cc_in[:].opt()],
                outs=[cc_out[:].opt()],
            )
            stats_g = spool.tile([h, 4], F32)
            nc.gpsimd.dma_start(out=stats_g[:], in_=cc_out[:])

            # ---- finalize norm scalars (all [h,1] f32) ----
            s1 = stats_g[:, 0:1]
            s2 = stats_g[:, 1:2]
            cc = stats_g[:, 2:3]
            tmp = spool.tile([h, 8], F32)
            cclamp = tmp[:, 0:1]
            cinv = tmp[:, 1:2]
            mean = tmp[:, 2:3]
            var = tmp[:, 3:4]
            rstd = tmp[:, 4:5]
            rsc = tmp[:, 5:6]
            sh = tmp[:, 6:7]
            scr = tmp[:, 7:8]
            nc.vector.tensor_scalar_max(cclamp, cc, 1.0)
            nc.vector.reciprocal(cinv, cclamp)
            nc.vector.tensor_mul(mean, s1, cinv)
            # var = S2*cinv - mean^2 * (2 - n_all*cinv)
            nc.vector.tensor_scalar(
                out=scr, in0=cinv, scalar1=-float(n_all), scalar2=2.0,
                op0=mult_op, op1=add_op,
            )
            nc.vector.tensor_mul(var, mean, mean)
            nc.vector.tensor_mul(var, var, scr)
            nc.vector.tensor_mul(scr, s2, cinv)
            nc.vector.tensor_sub(var, scr, var)
            epst = spool.tile([h, 1], F32)
            nc.vector.memset(epst[:], EPS)
            nc.scalar.activation(out=rstd, in_=var, func=sqrt_f, bias=epst[:])
            nc.vector.reciprocal(rstd, rstd)
            nc.vector.tensor_mul(rsc, rstd, gsc_sb[:])
            nc.vector.tensor_mul(sh, mean, rsc)
            nc.vector.tensor_sub(sh, gsh_sb[:], sh)

            # ---- out = (upd * rsc + sh) * mask ----
            out_sb = spool.tile([h, nodes], F32)
            nc.vector.tensor_scalar(
                out=out_sb[:], in0=upd[:], scalar1=rsc, scalar2=sh,
                op0=mult_op, op1=add_op,
            )
            nc.vector.tensor_mul(out_sb[:], out_sb[:], m_rep[:, 0:nodes])
            nc.sync.dma_start(out=out_ext[:], in_=out_sb[:])

    nc.compile()
    return nc


def prep_shards(atom_encode, atom_mask, dist_neighbors, edge_index,
                W0, b0, W1, b1, W2, b2, scale, shift,
                num_cores=NCORES, n_all=N, nodes=NODES, k=K, h=H):
    """Host-side layout prep: one in_map per core."""
    w0a = np.ascontiguousarray(W0[:h, :]).astype(BF)
    w0b = np.ascontiguousarray(W0[h : 2 * h, :]).astype(BF)
    w0d = W0[2 * h, :].astype(np.float32)
    w1 = np.ascontiguousarray(W1).astype(BF)
    w2 = np.ascontiguousarray(W2).astype(BF)
    seld = np.zeros((k, k * h), np.float32)
    seli = np.zeros((k, k * h), np.float32)
    for j in range(k):
        seld[j, j * h : (j + 1) * h] = w0d
        seli[j, j * h : (j + 1) * h] = NEG_BIG
    seld = seld.astype(BF)
    seli = seli.astype(BF)
    ident = np.eye(h, dtype=np.float32).astype(BF)
    onesk = np.ones((k, h), np.float32).astype(BF)
    common = dict(
        w0a=w0a, w0b=w0b, w1=w1, w2=w2, seld=seld, seli=seli,
        ident=ident, onesk=onesk,
        b0=np.ascontiguousarray(b0.astype(np.float32)).reshape(h, 1),
        b1=np.ascontiguousarray(b1.astype(np.float32)).reshape(h, 1),
        b2=np.ascontiguousarray(b2.astype(np.float32)).reshape(h, 1),
        gscale=np.ascontiguousarray(scale.astype(np.float32)).reshape(h, 1),
        gshift=np.ascontiguousarray(shift.astype(np.float32)).reshape(h, 1),
    )
    in_maps = []
    for core in range(num_cores):
        b, half = core // 2, core % 2
        off = half * nodes
        # rotate sample so own nodes come first
        encT = np.roll(atom_encode[b].T, -off, axis=1).astype(np.float32)
        maskr = np.roll(atom_mask[b], -off).reshape(1, n_all).astype(np.float32)
        dist32 = np.ascontiguousarray(
            dist_neighbors[b, off : off + nodes, :].T
        ).astype(BF)
        ei = edge_index[b, off : off + nodes, :]  # (nodes, k) int32
        inv32 = np.ascontiguousarray((ei.T == -1)).astype(np.float32).astype(BF)
        idx = np.where(ei < 0, 0, (ei - off) % n_all).astype(np.int16).T  # (k, nodes)
        # dma_gather wrap: idx j of block kb at [j%16, kb*(nodes//16) + j//16]
        wrapped = np.zeros((16, k * (nodes // 16)), np.int16)
        for kb in range(k):
            wrapped[:, kb * (nodes // 16) : (kb + 1) * (nodes // 16)] = (
                idx[kb].reshape(nodes // 16, 16).T
            )
        idx16 = np.tile(wrapped, (8, 1))  # replicate to 128 partitions
        in_maps.append(
            dict(
                encT=np.ascontiguousarray(encT),
                maskr=maskr,
                dist32=dist32,
                inv32=inv32,
                idx16=np.ascontiguousarray(idx16),
                **common,
            )
        )
    return in_maps


_CACHED_NC = None


def kernel(atom_encode, atom_mask, dist_neighbors, edge_index,
           W0, b0, W1, b1, W2, b2, scale, shift):
    global _CACHED_NC
    atom_encode = np.asarray(atom_encode)
    atom_mask = np.asarray(atom_mask)
    dist_neighbors = np.asarray(dist_neighbors)
    edge_index = np.asarray(edge_index)
    in_maps = prep_shards(
        atom_encode, atom_mask, dist_neighbors, edge_index,
        np.asarray(W0), np.asarray(b0), np.asarray(W1), np.asarray(b1),
        np.asarray(W2), np.asarray(b2), np.asarray(scale), np.asarray(shift),
    )
    if _CACHED_NC is None:
        _CACHED_NC = build_nc()
    res = run_bass_kernel_spmd(_CACHED_NC, in_maps, core_ids=list(range(NCORES)))
    out = np.empty((B, N, H), np.float32)
    for core in range(NCORES):
        b, half = core // 2, core % 2
        out[b, half * NODES : (half + 1) * NODES, :] = res.results[core]["out"].T
    return (out, atom_mask, dist_neighbors, edge_index)


# revision 18
# speedup vs baseline: 1.0049x; 1.0049x over previous
"""Trainium2 Bass kernel for AtomMPNN (gnn_message_passing).

Full inputs in, full outputs out. Internally: 8 NeuronCores, one (sample,
node-half) shard per core. See build_nc() for the device-side pipeline.

Self-contained: hardcodes all shapes; only imports the concourse runtime.
"""

import sys

sys.path.insert(0, "/opt/trn_rl_repo")

import numpy as np
import ml_dtypes

import concourse.bass as bass
import concourse.tile as tile
from concourse import bacc, mybir
from concourse import library_config
import concourse.tile_sem_assignment as _tsa
from concourse import bass_isa as _bass_isa

# Tile assigns SWDGE completion-sem lanes (DMASW0-7) round-robin, ignoring
# the instruction's SWDGE queue. With multiple queues, transfers on different
# queues complete out of order, so a lane must never mix queues. Patch the
# tick assigner to derive the lane from the queue: lane = 2*queue + rr.
_orig_assign_tick = _tsa.TileClockTick._assign_tick


def _queue_aware_assign_tick(self, inst):
    if (
        isinstance(inst, _tsa.DMAInst)
        and inst.engine == mybir.EngineType.Pool
        and not isinstance(inst, _bass_isa.UserSyncedRemoteDMADescs)
    ):
        qn = int(getattr(inst, "queue_num", 0) or 0)
        rr = getattr(self, "_per_queue_rr", None)
        if rr is None:
            rr = self._per_queue_rr = {}
        i = rr.get(qn, 0)
        rr[qn] = i ^ 1
        self.next_sw_dma_idx = 2 * qn + i
    return _orig_assign_tick(self, inst)


# Only needed when num_swdge_queues > 1; harmless but disabled for the
# single-queue configuration (original round-robin is queue-consistent).
_ENABLE_QUEUE_AWARE_LANES = False
if _ENABLE_QUEUE_AWARE_LANES:
    _tsa.TileClockTick._assign_tick = _queue_aware_assign_tick
from concourse.bass_utils import run_bass_kernel_spmd

B, N, K, H = 4, 4096, 32, 128
NODES = N // 2  # own nodes per core
NCORES = 8
EPS = 1e-5
NEG_BIG = -30000.0  # forces gelu(x) == 0 for padded edges

F32 = mybir.dt.float32
BF16 = mybir.dt.bfloat16
I16 = mybir.dt.int16

BF = ml_dtypes.bfloat16


def build_nc(num_cores=NCORES, n_all=N, nodes=NODES, k=K, h=H):
    """Build the SPMD Bass graph (same program on every core)."""
    nc = bacc.Bacc(
        "TRN2",
        target_bir_lowering=False,
        debug=False,
        num_devices=num_cores,
        num_swdge_queues=1,
    )

    # ---- DRAM parameters (per-core shards; host-prepped layouts) ----
    encT = nc.declare_dram_parameter("encT", [h, n_all], F32, isOutput=False)
    maskr = nc.declare_dram_parameter("maskr", [1, n_all], F32, isOutput=False)
    dist32 = nc.declare_dram_parameter("dist32", [k, nodes], BF16, isOutput=False)
    inv32 = nc.declare_dram_parameter("inv32", [k, nodes], BF16, isOutput=False)
    idx16 = nc.declare_dram_parameter("idx16", [128, k * (nodes // 16)], I16, isOutput=False)
    w0a = nc.declare_dram_parameter("w0a", [h, h], BF16, isOutput=False)
    w0b = nc.declare_dram_parameter("w0b", [h, h], BF16, isOutput=False)
    w1 = nc.declare_dram_parameter("w1", [h, h], BF16, isOutput=False)
    w2 = nc.declare_dram_parameter("w2", [h, h], BF16, isOutput=False)
    seld = nc.declare_dram_parameter("seld", [k, k * h], BF16, isOutput=False)
    seli = nc.declare_dram_parameter("seli", [k, k * h], BF16, isOutput=False)
    ident = nc.declare_dram_parameter("ident", [h, h], BF16, isOutput=False)
    onesk = nc.declare_dram_parameter("onesk", [k, h], BF16, isOutput=False)
    b0 = nc.declare_dram_parameter("b0", [h, 1], F32, isOutput=False)
    b1 = nc.declare_dram_parameter("b1", [h, 1], F32, isOutput=False)
    b2 = nc.declare_dram_parameter("b2", [h, 1], F32, isOutput=False)
    gscale = nc.declare_dram_parameter("gscale", [h, 1], F32, isOutput=False)
    gshift = nc.declare_dram_parameter("gshift", [h, 1], F32, isOutput=False)
    out_ext = nc.declare_dram_parameter("out", [h, nodes], F32, isOutput=True)

    ew = min(512, nodes)  # matmul free-dim slice width
    nsl = nodes // ew  # slices per block
    nchunk = n_all // 128  # 128-col chunks for the gather table
    pair_groups = [[2 * i, 2 * i + 1] for i in range(num_cores // 2)]
    gelu = mybir.ActivationFunctionType.Gelu
    identity_f = mybir.ActivationFunctionType.Identity
    square_f = mybir.ActivationFunctionType.Square
    sqrt_f = mybir.ActivationFunctionType.Sqrt
    add_op = mybir.AluOpType.add
    mult_op = mybir.AluOpType.mult

    with tile.TileContext(nc) as tc:
        with (
            tc.tile_pool(name="const", bufs=1) as cpool,
            tc.tile_pool(name="stat", bufs=1) as spool,
            tc.tile_pool(name="work", bufs=3) as wpool,
            tc.tile_pool(name="psum", bufs=2, space="PSUM") as ppool,
            tc.tile_pool(name="dram", bufs=1, space="DRAM") as dpool,
        ):
            # dma_gather lives in the 'mlp' GPSIMD ext-isa library
            nc.gpsimd.load_library(library_config.mlp)

            # ---- load constants / weights ----
            w0a_sb = cpool.tile([h, h], BF16)
            w0b_sb = cpool.tile([h, h], BF16)
            w1_sb = cpool.tile([h, h], BF16)
            w2_sb = cpool.tile([h, h], BF16)
            id_sb = cpool.tile([h, h], BF16)
            seld_sb = cpool.tile([k, k * h], BF16)
            seli_sb = cpool.tile([k, k * h], BF16)
            onesk_sb = cpool.tile([k, h], BF16)
            b0_sb = cpool.tile([h, 1], F32)
            b1_sb = cpool.tile([h, 1], F32)
            b2_sb = cpool.tile([h, 1], F32)
            gsc_sb = cpool.tile([h, 1], F32)
            gsh_sb = cpool.tile([h, 1], F32)
            for dst, src in [
                (w0a_sb, w0a), (w0b_sb, w0b), (w1_sb, w1), (w2_sb, w2),
                (id_sb, ident), (seld_sb, seld), (seli_sb, seli),
                (onesk_sb, onesk), (b0_sb, b0), (b1_sb, b1), (b2_sb, b2),
                (gsc_sb, gscale), (gsh_sb, gshift),
            ]:
                nc.sync.dma_start(out=dst[:], in_=src[:])

            # ---- load per-core data ----
            encT_sb = spool.tile([h, n_all], F32)
            nc.sync.dma_start(out=encT_sb[:], in_=encT[:])
            dist_sb = spool.tile([k, nodes], BF16)
            nc.sync.dma_start(out=dist_sb[:], in_=dist32[:])
            inv_sb = spool.tile([k, nodes], BF16)
            nc.sync.dma_start(out=inv_sb[:], in_=inv32[:])
            idx_sb = spool.tile([128, k * (nodes // 16)], I16)
            nc.sync.dma_start(out=idx_sb[:], in_=idx16[:])
            # mask broadcast to all 128 partitions (stride-0 partition read)
            m_rep = spool.tile([128, n_all], F32)
            mask_bcast = bass.AP(
                tensor=maskr.tensor if hasattr(maskr, "tensor") else maskr[:].tensor,
                offset=maskr[:].offset,
                ap=[[0, 128]] + list(maskr[:].ap[1:]),
            )
            nc.gpsimd.dma_start(out=m_rep[:], in_=mask_bcast)

            # ---- masked features (bf16): enc_m = encT * mask ----
            enc_m = spool.tile([h, n_all], BF16)
            nc.vector.tensor_mul(enc_m[:], encT_sb[:], m_rep[:])

            # ---- gather table P[i,:] = enc_m[:,i] @ W0a  (node-major rows) ----
            # node i -> partition i%128, stripe i//128 (256B bf16 stripes)
            ptab = spool.tile([128, nchunk * h], BF16)
            for q in range(nchunk):
                ps = ppool.tile([128, 2048], F32, tag="ps")
                nc.tensor.matmul(
                    ps[:, 0:h],
                    lhsT=enc_m[:, q * 128 : (q + 1) * 128],
                    rhs=w0a_sb[:],
                    start=True,
                    stop=True,
                )
                nc.vector.tensor_copy(
                    out=ptab[:, q * h : (q + 1) * h], in_=ps[:, 0:h]
                )

            # ---- self term S'' = W0b^T @ enc_m[:, :nodes] + b0 (bf16) ----
            s2_sb = spool.tile([h, nodes], BF16)
            ps_s = ppool.tile([128, 2048], F32, tag="ps")
            for q in range(nsl):
                nc.tensor.matmul(
                    ps_s[:, q * ew : (q + 1) * ew],
                    lhsT=w0b_sb[:],
                    rhs=enc_m[:, q * ew : (q + 1) * ew],
                    start=True,
                    stop=True,
                )
            nc.scalar.activation(
                out=s2_sb[:], in_=ps_s[:, 0:nodes], func=identity_f, bias=b0_sb[:]
            )

            # ---- valid-neighbor count, replicated: vcnt_inv[c,n] = 1/max(k - sum_j inv, 1)
            ps_v = ppool.tile([128, 2048], F32, tag="ps")
            for q in range(nsl):
                nc.tensor.matmul(
                    ps_v[:, q * ew : (q + 1) * ew],
                    lhsT=onesk_sb[:],
                    rhs=inv_sb[:, q * ew : (q + 1) * ew],
                    start=True,
                    stop=True,
                )
            vcnt = spool.tile([h, nodes], F32)
            nc.vector.tensor_scalar(
                out=vcnt[:], in0=ps_v[:, 0:nodes], scalar1=-1.0, scalar2=float(k),
                op0=mult_op, op1=add_op,
            )
            nc.vector.tensor_scalar_max(vcnt[:], vcnt[:], 1.0)
            vinv = spool.tile([h, nodes], F32)
            nc.vector.reciprocal(vinv[:], vcnt[:])

            # ---- edge blocks: block k' = neighbor slot k' for all own nodes ----
            msum = spool.tile([h, nodes], BF16)
            for kb in range(k):
                g_t = wpool.tile([128, 1, nodes], BF16, tag="g")
                nc.gpsimd.dma_gather(
                    out_ap=g_t[:],
                    in_ap=ptab[:],
                    idxs_ap=idx_sb[:, kb * (nodes // 16) : (kb + 1) * (nodes // 16)],
                    num_idxs=nodes,
                    num_idxs_reg=nodes,
                    elem_size=h,
                    transpose=True,
                    sbuf_tokens_per_rank=128,
                    sbuf_free_dim_per_rank=2 * h,
                    single_packet=False,
                )
                # t1 = gathered + self
                t1 = wpool.tile([h, nodes], BF16, tag="t1")
                nc.vector.tensor_add(t1[:], g_t[:, 0, :], s2_sb[:])
                # a0 = t1 + dist*w0d  (+b0 already inside s2)
                ps_a = ppool.tile([128, 2048], F32, tag="ps")
                for q in range(nsl):
                    sl = slice(q * ew, (q + 1) * ew)
                    nc.tensor.matmul(
                        ps_a[:, sl],
                        lhsT=seld_sb[:, kb * h : (kb + 1) * h],
                        rhs=dist_sb[:, sl],
                        start=True,
                        stop=False,
                    )
                for q in range(nsl):
                    sl = slice(q * ew, (q + 1) * ew)
                    nc.tensor.matmul(
                        ps_a[:, sl], lhsT=id_sb[:], rhs=t1[:, sl],
                        start=False, stop=True,
                    )
                h0 = wpool.tile([h, nodes], BF16, tag="h0")
                nc.scalar.activation(out=h0[:], in_=ps_a[:, 0:nodes], func=gelu)
                # layer 1
                ps_b = ppool.tile([128, 2048], F32, tag="ps")
                for q in range(nsl):
                    sl = slice(q * ew, (q + 1) * ew)
                    nc.tensor.matmul(
                        ps_b[:, sl], lhsT=w1_sb[:], rhs=h0[:, sl],
                        start=True, stop=True,
                    )
                h1 = wpool.tile([h, nodes], BF16, tag="h1")
                nc.scalar.activation(
                    out=h1[:], in_=ps_b[:, 0:nodes], func=gelu, bias=b1_sb[:]
                )
                # layer 2 (+ -30000 * invalid so gelu masks padded edges)
                ps_c = ppool.tile([128, 2048], F32, tag="ps")
                for q in range(nsl):
                    sl = slice(q * ew, (q + 1) * ew)
                    nc.tensor.matmul(
                        ps_c[:, sl], lhsT=w2_sb[:], rhs=h1[:, sl],
                        start=True, stop=False,
                    )
                for q in range(nsl):
                    sl = slice(q * ew, (q + 1) * ew)
                    nc.tensor.matmul(
                        ps_c[:, sl],
                        lhsT=seli_sb[:, kb * h : (kb + 1) * h],
                        rhs=inv_sb[:, sl],
                        start=False,
                        stop=True,
                    )
                mk = wpool.tile([h, nodes], BF16, tag="mk")
                nc.scalar.activation(
                    out=mk[:], in_=ps_c[:, 0:nodes], func=gelu, bias=b2_sb[:]
                )
                if kb == 0:
                    nc.vector.tensor_copy(out=msum[:], in_=mk[:])
                else:
                    nc.vector.tensor_add(msum[:], msum[:], mk[:])

            # ---- node phase: upd = (enc + msum/vcnt) * mask ----
            msum_f = spool.tile([h, nodes], F32)
            nc.vector.tensor_copy(out=msum_f[:], in_=msum[:])
            upd = spool.tile([h, nodes], F32)
            nc.vector.tensor_mul(upd[:], msum_f[:], vinv[:])
            nc.vector.tensor_add(upd[:], upd[:], encT_sb[:, 0:nodes])
            nc.vector.tensor_mul(upd[:], upd[:], m_rep[:, 0:nodes])

            # ---- partial stats: S1 = sum(upd), S2 = sum(upd^2), C = sum(mask)
            stats = spool.tile([h, 4], F32)
            nc.vector.tensor_reduce(
                out=stats[:, 0:1], in_=upd[:], axis=mybir.AxisListType.X, op=add_op
            )
            sq_trash = spool.tile([h, nodes], BF16)
            nc.scalar.activation(
                out=sq_trash[:], in_=upd[:], func=square_f,
                accum_out=stats[:, 1:2],
            )
            nc.vector.tensor_reduce(
                out=stats[:, 2:3], in_=m_rep[:, 0:nodes],
                axis=mybir.AxisListType.X, op=add_op,
            )
            nc.vector.memset(stats[:, 3:4], 0.0)

            # ---- pair all-reduce of stats (both halves of the sample) ----
            cc_in = dpool.tile([h, 4], F32)
            cc_out = dpool.tile([h, 4], F32)
            nc.sync.dma_start(out=cc_in[:], in_=stats[:])
            nc.gpsimd.collective_compute(
                "AllReduce",
                mybir.AluOpType.add,
                replica_groups=pair_groups,
                ins=[cc_in[:].opt()],
                outs=[cc_out[:].opt()],
            )
            stats_g = spool.tile([h, 4], F32)
            nc.sync.dma_start(out=stats_g[:], in_=cc_out[:])

            # ---- finalize norm scalars (all [h,1] f32) ----
            s1 = stats_g[:, 0:1]
            s2 = stats_g[:, 1:2]
            cc = stats_g[:, 2:3]
            tmp = spool.tile([h, 8], F32)
            cclamp = tmp[:, 0:1]
            cinv = tmp[:, 1:2]
            mean = tmp[:, 2:3]
            var = tmp[:, 3:4]
            rstd = tmp[:, 4:5]
            rsc = tmp[:, 5:6]
            sh = tmp[:, 6:7]
            scr = tmp[:, 7:8]
            nc.vector.tensor_scalar_max(cclamp, cc, 1.0)
            nc.vector.reciprocal(cinv, cclamp)
            nc.vector.tensor_mul(mean, s1, cinv)
            # var = S2*cinv - mean^2 * (2 - n_all*cinv)
            nc.vector.tensor_scalar(
                out=scr, in0=cinv, scalar1=-float(n_all), scalar2=2.0,
                op0=mult_op, op1=add_op,
            )
            nc.vector.tensor_mul(var, mean, mean)
            nc.vector.tensor_mul(var, var, scr)
            nc.vector.tensor_mul(scr, s2, cinv)
            nc.vector.tensor_sub(var, scr, var)
            epst = spool.tile([h, 1], F32)
            nc.vector.memset(epst[:], EPS)
            nc.scalar.activation(out=rstd, in_=var, func=sqrt_f, bias=epst[:])
            nc.vector.reciprocal(rstd, rstd)
            nc.vector.tensor_mul(rsc, rstd, gsc_sb[:])
            nc.vector.tensor_mul(sh, mean, rsc)
            nc.vector.tensor_sub(sh, gsh_sb[:], sh)

            # ---- out = (upd * rsc + sh) * mask ----
            out_sb = spool.tile([h, nodes], F32)
            nc.vector.tensor_scalar(
                out=out_sb[:], in0=upd[:], scalar1=rsc, scalar2=sh,
                op0=mult_op, op1=add_op,
            )
            nc.vector.tensor_mul(out_sb[:], out_sb[:], m_rep[:, 0:nodes])
            nc.sync.dma_start(out=out_ext[:], in_=out_sb[:])

    nc.compile()
    return nc


def prep_shards(atom_encode, atom_mask, dist_neighbors, edge_index,
                W0, b0, W1, b1, W2, b2, scale, shift,
                num_cores=NCORES, n_all=N, nodes=NODES, k=K, h=H):
    """Host-side layout prep: one in_map per core."""
    w0a = np.ascontiguousarray(W0[:h, :]).astype(BF)
    w0b = np.ascontiguousarray(W0[h : 2 * h, :]).astype(BF)
    w0d = W0[2 * h, :].astype(np.float32)
    w1 = np.ascontiguousarray(W1).astype(BF)
    w2 = np.ascontiguousarray(W2).astype(BF)
    seld = np.zeros((k, k * h), np.float32)
    seli = np.zeros((k, k * h), np.float32)
    for j in range(k):
        seld[j, j * h : (j + 1) * h] = w0d
        seli[j, j * h : (j + 1) * h] = NEG_BIG
    seld = seld.astype(BF)
    seli = seli.astype(BF)
    ident = np.eye(h, dtype=np.float32).astype(BF)
    onesk = np.ones((k, h), np.float32).astype(BF)
    common = dict(
        w0a=w0a, w0b=w0b, w1=w1, w2=w2, seld=seld, seli=seli,
        ident=ident, onesk=onesk,
        b0=np.ascontiguousarray(b0.astype(np.float32)).reshape(h, 1),
        b1=np.ascontiguousarray(b1.astype(np.float32)).reshape(h, 1),
        b2=np.ascontiguousarray(b2.astype(np.float32)).reshape(h, 1),
        gscale=np.ascontiguousarray(scale.astype(np.float32)).reshape(h, 1),
        gshift=np.ascontiguousarray(shift.astype(np.float32)).reshape(h, 1),
    )
    in_maps = []
    for core in range(num_cores):
        b, half = core // 2, core % 2
        off = half * nodes
        # rotate sample so own nodes come first
        encT = np.roll(atom_encode[b].T, -off, axis=1).astype(np.float32)
        maskr = np.roll(atom_mask[b], -off).reshape(1, n_all).astype(np.float32)
        dist32 = np.ascontiguousarray(
            dist_neighbors[b, off : off + nodes, :].T
        ).astype(BF)
        ei = edge_index[b, off : off + nodes, :]  # (nodes, k) int32
        inv32 = np.ascontiguousarray((ei.T == -1)).astype(np.float32).astype(BF)
        idx = np.where(ei < 0, 0, (ei - off) % n_all).astype(np.int16).T  # (k, nodes)
        # dma_gather wrap: idx j of block kb at [j%16, kb*(nodes//16) + j//16]
        wrapped = np.zeros((16, k * (nodes // 16)), np.int16)
        for kb in range(k):
            wrapped[:, kb * (nodes // 16) : (kb + 1) * (nodes // 16)] = (
                idx[kb].reshape(nodes // 16, 16).T
            )
        idx16 = np.tile(wrapped, (8, 1))  # replicate to 128 partitions
        in_maps.append(
            dict(
                encT=np.ascontiguousarray(encT),
                maskr=maskr,
                dist32=dist32,
                inv32=inv32,
                idx16=np.ascontiguousarray(idx16),
                **common,
            )
        )
    return in_maps


_CACHED_NC = None


def kernel(atom_encode, atom_mask, dist_neighbors, edge_index,
           W0, b0, W1, b1, W2, b2, scale, shift):
    global _CACHED_NC
    atom_encode = np.asarray(atom_encode)
    atom_mask = np.asarray(atom_mask)
    dist_neighbors = np.asarray(dist_neighbors)
    edge_index = np.asarray(edge_index)
    in_maps = prep_shards(
        atom_encode, atom_mask, dist_neighbors, edge_index,
        np.asarray(W0), np.asarray(b0), np.asarray(W1), np.asarray(b1),
        np.asarray(W2), np.asarray(b2), np.asarray(scale), np.asarray(shift),
    )
    if _CACHED_NC is None:
        _CACHED_NC = build_nc()
    res = run_bass_kernel_spmd(_CACHED_NC, in_maps, core_ids=list(range(NCORES)))
    out = np.empty((B, N, H), np.float32)
    for core in range(NCORES):
        b, half = core // 2, core % 2
        out[b, half * NODES : (half + 1) * NODES, :] = res.results[core]["out"].T
    return (out, atom_mask, dist_neighbors, edge_index)
